# revision 34
# baseline (speedup 1.0000x reference)
"""DetectionLoss Trainium2 Bass kernel, v3.

Data-parallel over batch: 2 images per core x 8 cores; host sums 18 partial
sums per core (npos is a global normalizer).

Every loss term is either (a) a reduction over the dense obj logits
(softplus), or (b) a function of values at the <=128 positive cells per
scale.  The cls logsumexp therefore does NOT need the dense cls tensor on
device: host-repack cls into per-cell records (pure relayout, like the v1
objreg records) and indirect-gather one 36-float row per (box, scale) -
obj, reg0..3, cls0..29.

v3 over v2:
  - ONE merged indirect gather (offset ap [128,3], out [128,3,36]): SWDGE
    descriptor generation costs 994ns fixed + 0.34ns/desc, so one op for
    384 rows beats three ops for 128 rows by ~2.1us of serial gpsimd time.
  - The box->key index chain runs on gpsimd itself (Pool ALU), so the
    gather issues with no cross-engine handoff; DVE reads gpsimd's keyf
    for the winner/min-label masks in parallel.
  - smooth-L1 chain also on gpsimd (idle after the gather) in parallel
    with DVE's cls-select and ACT's logsumexp.
  - final partials via ones-column matmul -> [1,18] PSUM -> single-
    descriptor DMA out (v2's [18,1] out burned 900ns generating 18
    descriptors on the sync sequencer).
  - single ACT table load (combined exp+ln set) patched post-compile.
"""

import numpy as np

import concourse.bass as bass
import concourse.tile as tile
from concourse import bacc, mybir
from concourse.bass_utils import run_bass_kernel_spmd

F32 = mybir.dt.float32
I32 = mybir.dt.int32
AF = mybir.ActivationFunctionType
OP = mybir.AluOpType
AX = mybir.AxisListType

B_TOT = 16
N_CORES = 8
B_SH = B_TOT // N_CORES
NBOX = 64
NP = B_SH * NBOX  # 128 partitions: (image, box)
C = 30
SCALES = [(80, 80), (40, 40), (20, 20)]
NREC = sum(B_SH * h * w for h, w in SCALES)  # 16800
BASES = [0, 12800, 16000]
RECW = 64  # obj, reg0..3, cls0..29, pad to 256B rows (aligned gather descriptors)
BIGL = 65536.0  # label offset for the min-label trick (exact in f32)
PADV = -200.0  # softplus(PADV) == 0 in f32
NPART = 18  # per scale s, cols 6s + [lse, clsval, sl1, obj, softplus, npos]

CLS_W, REG_W, OBJ_W = 1.0, 5.0, 1.0

# Pool (gpsimd) fails walrus ISA checks for tensor_tensor with broadcast
# APs, so the elementwise chains stay on DVE
CHAIN_ON_GPSIMD = False

_DBG = None  # set by test_debug.py to dump (recg, keyi)

# Pool partition-reduce measured 2.5us for [128,18] (plus library reloads);
# the PE ones-matmul finish is ~0.6us
FIN_ON_GPSIMD = False


def _bigt_const():
    ident = np.eye(128, dtype=np.float32)
    utri = np.triu(np.ones((128, 128), np.float32), 1)
    return np.concatenate([ident, utri], axis=1)  # [128, 256]


def _smalls_consts():
    """Constant columns 5:57 of the smalls input.  Columns 46:52 carry
    int32 grid constants bit-cast into the f32 array; the device reads
    them through an AP bitcast."""
    p = np.arange(128)
    bvec = (p >= NBOX).astype(np.float32)
    kc = np.zeros((128, 52), np.float32)
    for s, (h, w) in enumerate(SCALES):
        kc[:, 0 + s] = w
        kc[:, 3 + s] = h
    kc[:, 15:45] = np.arange(C, dtype=np.float32)[None, :]
    kc[:, 45] = 1.0  # ones column for the final partials matmul
    ki = np.zeros((128, 6), np.int32)
    for s, (h, w) in enumerate(SCALES):
        ki[:, 0 + s] = w
        ki[:, 3 + s] = (bvec * h * w).astype(np.int32) + BASES[s]
    kc[:, 46:52] = ki.view(np.float32)
    return kc


_SMALLS_KC = _smalls_consts()


def emit(tc: tile.TileContext, out_ap, ins):
    nc = tc.nc
    pools = []

    def mkpool(**kw):
        p = tc.alloc_tile_pool(**kw)
        pools.append(p)
        return p

    pool = mkpool(name="sb", bufs=1)
    kmps = mkpool(name="kmps", bufs=1, space="PSUM")
    fips = mkpool(name="fips", bufs=1, space="PSUM")

    # ---- input loads, spread across the three DMA-capable queues
    smalls = pool.tile([128, 64], F32, tag="smalls")
    nc.sync.dma_start(out=smalls[:], in_=ins["smalls"])
    bigt = pool.tile([128, 256], F32, tag="bigt")
    nc.gpsimd.dma_start(out=bigt[:], in_=ins["bigt"])
    objd = pool.tile([128, 132], F32, tag="objd")
    nc.scalar.dma_start(out=objd[:], in_=ins["objd"])

    ident = bigt[:, 0:128]
    utri = bigt[:, 128:256]
    btile = smalls[:, 0:4]
    labB = smalls[:, 4:5]
    kxy = smalls[:, 5:11].rearrange("p (c s) -> p c s", c=2)
    iota30 = smalls[:, 20:50]
    ones = smalls[:, 50:51]
    wvec_i = smalls[:, 51:54].bitcast(I32)
    koff_i = smalls[:, 54:57].bitcast(I32)

    stack = pool.tile([128, NPART], F32, tag="stack")
    nc.vector.memset(stack[:], 0.0)
    stv = stack[:].rearrange("p (s j) -> p s j", j=6)

    ce = nc.gpsimd if CHAIN_ON_GPSIMD else nc.vector

    # ---- box -> cell key per scale.  floor via round(x - 0.5) fused into
    # the i32-out cast; the reference's clamps are provably no-ops for
    # f32 coords in [0, 1): x*W never rounds up to W and round(x*W - 0.5)
    # stays within [0, W-1].  Key arithmetic in int32 (no float round-trip).
    gr = pool.tile([NP, 2, 3], F32, tag="gr")
    ce.tensor_tensor(
        out=gr[:], in0=btile[:, 0:2, None].to_broadcast([NP, 2, 3]), in1=kxy, op=OP.mult
    )
    gi = pool.tile([NP, 2, 3], I32, tag="gi")
    ce.tensor_scalar(out=gi[:], in0=gr[:], scalar1=-0.5, scalar2=None, op0=OP.add)
    keyi = pool.tile([NP, 3], I32, tag="keyi")
    ce.tensor_tensor(out=keyi[:], in0=gi[:, 1, :], in1=wvec_i, op=OP.mult)
    ce.tensor_add(keyi[:], keyi[:], gi[:, 0, :])
    ce.tensor_add(keyi[:], keyi[:], koff_i)
    keyf = pool.tile([NP, 3], F32, tag="keyf")
    ce.tensor_copy(out=keyf[:], in_=keyi[:])

    # ---- record gathers: 36-float row per (box, scale).  One gather per
    # scale: multi-offset-per-partition indirect DMAs generate garbled
    # addresses on hardware (verified empirically), so three ops it is.
    recg = pool.tile([NP, 3, RECW], F32, tag="recg")
    for s in range(3):
        nc.gpsimd.indirect_dma_start(
            out=recg[:, s, :],
            out_offset=None,
            in_=ins["rec"],
            in_offset=bass.IndirectOffsetOnAxis(ap=keyi[:, s : s + 1], axis=0),
        )

    if _DBG is not None:
        dbg, dbgk = _DBG
        nc.sync.dma_start(out=dbg, in_=recg[:].rearrange("p s r -> p (s r)"))
        nc.sync.dma_start(out=dbgk, in_=keyi[:])

    # ---- key/label row matrices: PE transpose of broadcast columns
    kl = kmps.tile([128, 512], F32, tag="kl")
    klv = kl[:].rearrange("p (s q) -> p s q", s=4)
    for s in range(3):
        nc.tensor.transpose(
            out=kl[:, 128 * s : 128 * (s + 1)],
            in_=keyf[:, s : s + 1].to_broadcast([128, 128]),
            identity=ident,
        )
    nc.tensor.transpose(out=kl[:, 384:512], in_=labB.to_broadcast([128, 128]), identity=ident)

    # ---- obj softplus over all cells: exp now, ln(1+x) with accum later
    obje = pool.tile([128, 132], F32, tag="obje")
    nc.scalar.activation(out=obje[:], in_=objd[:], func=AF.Exp)
    for s, (a, b) in enumerate([(0, 100), (100, 125), (125, 132)]):
        objl = pool.tile([128, b - a], F32, tag=f"objl{s}")
        nc.scalar.activation(
            out=objl[:], in_=obje[:, a:b], func=AF.Ln, bias=1.0,
            accum_out=stack[:, 6 * s + 4 : 6 * s + 5],
        )

    # ---- winners (last box per cell) + min same-cell label, all 3 scales.
    # win path first: the per-scale blocks need win3 for everything but the
    # cls column; minlab only gates the (late) selm select.
    eqm3 = pool.tile([128, 3, 128], F32, tag="eqm3")
    nc.vector.tensor_tensor(
        out=eqm3[:], in0=klv[:, 0:3, :], in1=keyf[:, :, None].to_broadcast([128, 3, 128]),
        op=OP.is_equal,
    )
    lose3 = pool.tile([128, 3, 128], F32, tag="lose3")
    nc.vector.tensor_tensor(
        out=lose3[:], in0=eqm3[:], in1=utri[:, None, :].to_broadcast([128, 3, 128]), op=OP.mult
    )
    losev = pool.tile([NP, 3], F32, tag="losev")
    nc.vector.tensor_reduce(out=losev[:], in_=lose3[:], axis=AX.X, op=OP.max)
    win3 = pool.tile([NP, 3], F32, tag="win3")
    nc.vector.tensor_scalar(
        out=win3[:], in0=losev[:], scalar1=-1.0, scalar2=1.0, op0=OP.mult, op1=OP.add
    )
    cnd3 = pool.tile([128, 3, 128], F32, tag="cnd3")
    nc.vector.scalar_tensor_tensor(
        out=cnd3[:], in0=eqm3[:], scalar=-BIGL,
        in1=kl[:, None, 384:512].to_broadcast([128, 3, 128]), op0=OP.mult, op1=OP.add,
    )
    minlab3 = pool.tile([NP, 3], F32, tag="minlab3")
    nc.vector.tensor_reduce(out=minlab3[:], in_=cnd3[:], axis=AX.X, op=OP.min)
    nc.vector.tensor_copy(out=stv[:, :, 5], in_=win3[:])  # npos columns

    # ---- per-scale post-processing, pipelined in the shadow of the next
    # scale's gather (each gather lands ~1.5us apart)
    expc = pool.tile([NP, 3, C], F32, tag="expc")
    sume = pool.tile([NP, 3], F32, tag="sume")
    lse3 = pool.tile([NP, 3], F32, tag="lse3")
    selm = pool.tile([NP, 3, C], F32, tag="selm")
    d12 = pool.tile([NP, 3, 4], F32, tag="d12")
    q12 = pool.tile([NP, 3, 4], F32, tag="q12")
    h12 = pool.tile([NP, 3, 4], F32, tag="h12")
    sl13 = pool.tile([NP, 3], F32, tag="sl13")
    clsv3 = pool.tile([NP, 3], F32, tag="clsv3")

    for s in range(3):
        rs = recg[:, s, :]
        wins = win3[:, s : s + 1]
        # ACT: logsumexp (exp with free-axis accum, then ln) and the
        # win-scaled stack columns that don't need DVE results (activation
        # with a per-partition scale AP does the masking multiply)
        nc.scalar.activation(
            out=expc[:, s, :], in_=rs[:, 5:35], func=AF.Exp,
            accum_out=sume[:, s : s + 1],
        )
        nc.scalar.activation(out=lse3[:, s : s + 1], in_=sume[:, s : s + 1], func=AF.Ln)
        # DVE: smooth-L1 (beta=1, coord mean, clamp 10)
        nc.vector.tensor_tensor(out=d12[:, s, :], in0=rs[:, 1:5], in1=btile, op=OP.subtract)
        nc.vector.scalar_tensor_tensor(
            out=d12[:, s, :], in0=d12[:, s, :], scalar=-1.0, in1=d12[:, s, :],
            op0=OP.mult, op1=OP.max,
        )
        nc.vector.tensor_scalar_min(q12[:, s, :], d12[:, s, :], 1.0)
        nc.vector.scalar_tensor_tensor(
            out=h12[:, s, :], in0=q12[:, s, :], scalar=-0.5, in1=d12[:, s, :],
            op0=OP.mult, op1=OP.add,
        )
        nc.vector.scalar_tensor_tensor(
            out=h12[:, s, :], in0=h12[:, s, :], scalar=1.0, in1=q12[:, s, :],
            op0=OP.mult, op1=OP.mult, accum_out=sl13[:, s : s + 1],
        )
        nc.vector.tensor_scalar(
            out=sl13[:, s : s + 1], in0=sl13[:, s : s + 1],
            scalar1=0.25, scalar2=10.0, op0=OP.mult, op1=OP.min,
        )
        # stack columns for this scale
        nc.vector.tensor_mul(stv[:, s, 0:1], lse3[:, s : s + 1], wins)
        nc.vector.tensor_mul(stv[:, s, 2:3], sl13[:, s : s + 1], wins)
        nc.vector.tensor_mul(stv[:, s, 3:4], rs[:, 0:1], wins)
        # cls logit at the min label: fused per-partition-scalar mask+mult
        nc.vector.scalar_tensor_tensor(
            out=selm[:, s, :], in0=iota30, scalar=minlab3[:, s : s + 1],
            in1=rs[:, 5:35], op0=OP.is_equal, op1=OP.mult,
        )
        nc.vector.tensor_reduce(out=clsv3[:, s : s + 1], in_=selm[:, s, :], axis=AX.X, op=OP.add)
        nc.vector.tensor_mul(stv[:, s, 1:2], clsv3[:, s : s + 1], wins)

    # ---- final: partition-reduce the stack on gpsimd (idle after gathers)
    # -> [1,18] -> single-descriptor DMA out
    if FIN_ON_GPSIMD:
        fin_sb = pool.tile([1, NPART], F32, tag="fin_sb")
        nc.gpsimd.tensor_reduce(out=fin_sb[:], in_=stack[:], axis=AX.C, op=OP.add)
    else:
        fin_ps = fips.tile([1, NPART], F32, tag="fin_ps")
        nc.tensor.matmul(out=fin_ps[:], lhsT=ones, rhs=stack[:], start=True, stop=True)
        fin_sb = pool.tile([1, NPART], F32, tag="fin_sb")
        nc.vector.tensor_copy(out=fin_sb[:], in_=fin_ps[:])
    nc.sync.dma_start(out=out_ap, in_=fin_sb[:])

    for p in reversed(pools):
        p.release()


def _patch_act_tables(nc):
    """Point every ACT table load at the combined exp+ln set and drop the
    redundant reloads (the greedy insertion pass ping-pongs between the
    exp-only and ln-only tables).  Loads are inserted after semaphore
    generation, so removal is safe."""
    tables = list(__import__("concourse.hw_specs", fromlist=["x"]).get_activation_tables(nc.m.arch).items())
    target = None
    for i, (name, funcs) in enumerate(tables):
        if AF.Exp in funcs and AF.Ln in funcs:
            target = i
            break
    if target is None:
        return
    first_seen = False
    for blk in nc.main_func.blocks:
        keep = []
        for inst in blk.instructions:
            if isinstance(inst, mybir.InstLoadActFuncSet):
                if not first_seen:
                    inst.act_func_set_id = target
                    first_seen = True
                    keep.append(inst)
                # drop later loads: one combined table serves every func
            else:
                keep.append(inst)
        blk.instructions[:] = keep


# ---------------------------------------------------------------------------
# host side
# ---------------------------------------------------------------------------

_CACHE = {}


def _build():
    if "nc" in _CACHE:
        return _CACHE["nc"]
    nc = bacc.Bacc(
        "TRN2",
        target_bir_lowering=False,
        debug=False,
        enable_asserts=False,
        num_devices=N_CORES,
    )
    bigt_h = nc.inline_tensor(_bigt_const(), name="cbig")
    ins = {
        "rec": nc.dram_tensor("rec", (NREC, RECW), F32, kind="ExternalInput").ap(),
        "objd": nc.dram_tensor("objd", (128, 132), F32, kind="ExternalInput").ap(),
        "smalls": nc.dram_tensor("smalls", (128, 64), F32, kind="ExternalInput").ap(),
        "bigt": bigt_h.ap(),
    }
    out = nc.dram_tensor("partials", (1, NPART), F32, kind="ExternalOutput").ap()

    with tile.TileContext(nc) as tc:
        emit(tc, out, ins)
    nc.compile()
    _patch_act_tables(nc)
    _CACHE["nc"] = nc
    return nc


def _prep_core(inputs, lo, hi):
    rec = np.zeros((NREC, RECW), np.float32)
    r0 = 0
    for s, (h, w) in enumerate(SCALES):
        hw = h * w
        n = B_SH * hw
        rec[r0 : r0 + n, 0] = np.asarray(inputs[f"obj_p{s}"][lo:hi]).reshape(n)
        rec[r0 : r0 + n, 1:5] = (
            np.asarray(inputs[f"reg_p{s}"][lo:hi]).reshape(B_SH, 4, hw).transpose(0, 2, 1).reshape(n, 4)
        )
        rec[r0 : r0 + n, 5:35] = (
            np.asarray(inputs[f"cls_p{s}"][lo:hi]).reshape(B_SH, C, hw).transpose(0, 2, 1).reshape(n, C)
        )
        r0 += n

    objd = np.empty((128, 132), np.float32)
    objd[:, 0:100] = np.asarray(inputs["obj_p0"][lo:hi]).reshape(128, 100)
    objd[:, 100:125] = np.asarray(inputs["obj_p1"][lo:hi]).reshape(128, 25)
    z = np.full(896, PADV, np.float32)
    z[:800] = np.asarray(inputs["obj_p2"][lo:hi]).reshape(800)
    objd[:, 125:132] = z.reshape(128, 7)

    smalls = np.zeros((128, 64), np.float32)
    smalls[:, 0:4] = np.asarray(inputs["boxes"][lo:hi]).reshape(128, 4)
    smalls[:, 4] = np.asarray(inputs["labels"][lo:hi]).reshape(128).astype(np.float32) + BIGL
    smalls[:, 5:57] = _SMALLS_KC
    return {"rec": rec, "objd": objd, "smalls": smalls}


def combine_partials(parts):
    """parts: [n_cores, 18] -> final [4] losses."""
    tot = np.asarray(parts, np.float64).sum(axis=0)
    cls_sum = reg_sum = obj_sum = 0.0
    for s, (h, w) in enumerate(SCALES):
        b = 6 * s
        lse, val, sl1, obj, sp, npos = tot[b : b + 6]
        npos = max(npos, 1.0)
        cls_sum += (lse - val) / npos * CLS_W
        reg_sum += sl1 / npos * REG_W
        obj_sum += (sp - obj) / (B_TOT * h * w) * OBJ_W
    cls_sum /= len(SCALES)
    reg_sum /= len(SCALES)
    obj_sum /= len(SCALES)
    total = cls_sum + reg_sum + obj_sum
    return np.array([total, cls_sum, reg_sum, obj_sum], np.float32)


TRACE = False
LAST_RESULT = None


def kernel(**inputs):
    global LAST_RESULT
    nc = _build()
    in_maps = [_prep_core(inputs, c * B_SH, (c + 1) * B_SH) for c in range(N_CORES)]
    res = run_bass_kernel_spmd(
        nc, in_maps, core_ids=list(range(N_CORES)), trace=TRACE
    )
    LAST_RESULT = res
    parts = np.stack([np.asarray(r["partials"]).reshape(NPART) for r in res.results])
    return combine_partials(parts)


# revision 35
# speedup vs baseline: 1.0890x; 1.0890x over previous
"""DetectionLoss Trainium2 Bass kernel, v3.

Data-parallel over batch: 2 images per core x 8 cores; host sums 18 partial
sums per core (npos is a global normalizer).

Every loss term is either (a) a reduction over the dense obj logits
(softplus), or (b) a function of values at the <=128 positive cells per
scale.  The cls logsumexp therefore does NOT need the dense cls tensor on
device: host-repack cls into per-cell records (pure relayout, like the v1
objreg records) and indirect-gather one 36-float row per (box, scale) -
obj, reg0..3, cls0..29.

v3 over v2:
  - ONE merged indirect gather (offset ap [128,3], out [128,3,36]): SWDGE
    descriptor generation costs 994ns fixed + 0.34ns/desc, so one op for
    384 rows beats three ops for 128 rows by ~2.1us of serial gpsimd time.
  - The box->key index chain runs on gpsimd itself (Pool ALU), so the
    gather issues with no cross-engine handoff; DVE reads gpsimd's keyf
    for the winner/min-label masks in parallel.
  - smooth-L1 chain also on gpsimd (idle after the gather) in parallel
    with DVE's cls-select and ACT's logsumexp.
  - final partials via ones-column matmul -> [1,18] PSUM -> single-
    descriptor DMA out (v2's [18,1] out burned 900ns generating 18
    descriptors on the sync sequencer).
  - single ACT table load (combined exp+ln set) patched post-compile.
"""

import numpy as np

import concourse.bass as bass
import concourse.tile as tile
from concourse import bacc, mybir
from concourse.bass_utils import run_bass_kernel_spmd

F32 = mybir.dt.float32
I32 = mybir.dt.int32
AF = mybir.ActivationFunctionType
OP = mybir.AluOpType
AX = mybir.AxisListType

B_TOT = 16
N_CORES = 8
B_SH = B_TOT // N_CORES
NBOX = 64
NP = B_SH * NBOX  # 128 partitions: (image, box)
C = 30
SCALES = [(80, 80), (40, 40), (20, 20)]
NREC = sum(B_SH * h * w for h, w in SCALES)  # 16800
BASES = [0, 12800, 16000]
RECW = 36  # obj, reg0..3, cls0..29, pad
BIGL = 65536.0  # label offset for the min-label trick (exact in f32)
PADV = -200.0  # softplus(PADV) == 0 in f32
NPART = 18  # per scale s, cols 6s + [lse, clsval, sl1, obj, softplus, npos]

CLS_W, REG_W, OBJ_W = 1.0, 5.0, 1.0

# Pool (gpsimd) fails walrus ISA checks for tensor_tensor with broadcast
# APs, so the elementwise chains stay on DVE
CHAIN_ON_GPSIMD = False

_DBG = None  # set by test_debug.py to dump (recg, keyi)

# Pool partition-reduce measured 2.5us for [128,18] (plus library reloads);
# the PE ones-matmul finish is ~0.6us
FIN_ON_GPSIMD = False


def _bigt_const():
    ident = np.eye(128, dtype=np.float32)
    utri = np.triu(np.ones((128, 128), np.float32), 1)
    return np.concatenate([ident, utri], axis=1)  # [128, 256]


def _smalls_consts():
    """Constant columns 5:57 of the smalls input.  Columns 46:52 carry
    int32 grid constants bit-cast into the f32 array; the device reads
    them through an AP bitcast."""
    p = np.arange(128)
    bvec = (p >= NBOX).astype(np.float32)
    kc = np.zeros((128, 52), np.float32)
    for s, (h, w) in enumerate(SCALES):
        kc[:, 0 + s] = w
        kc[:, 3 + s] = h
    kc[:, 15:45] = np.arange(C, dtype=np.float32)[None, :]
    kc[:, 45] = 1.0  # ones column for the final partials matmul
    ki = np.zeros((128, 6), np.int32)
    for s, (h, w) in enumerate(SCALES):
        ki[:, 0 + s] = w
        ki[:, 3 + s] = (bvec * h * w).astype(np.int32) + BASES[s]
    kc[:, 46:52] = ki.view(np.float32)
    return kc


_SMALLS_KC = _smalls_consts()


def emit(tc: tile.TileContext, out_ap, ins):
    nc = tc.nc
    pools = []

    def mkpool(**kw):
        p = tc.alloc_tile_pool(**kw)
        pools.append(p)
        return p

    pool = mkpool(name="sb", bufs=1)
    kmps = mkpool(name="kmps", bufs=1, space="PSUM")
    fips = mkpool(name="fips", bufs=1, space="PSUM")

    # ---- input loads, spread across the three DMA-capable queues
    smalls = pool.tile([128, 64], F32, tag="smalls")
    nc.sync.dma_start(out=smalls[:], in_=ins["smalls"])
    bigt = pool.tile([128, 256], F32, tag="bigt")
    nc.gpsimd.dma_start(out=bigt[:], in_=ins["bigt"])
    objd = pool.tile([128, 132], F32, tag="objd")
    nc.scalar.dma_start(out=objd[:], in_=ins["objd"])

    ident = bigt[:, 0:128]
    utri = bigt[:, 128:256]
    btile = smalls[:, 0:4]
    labB = smalls[:, 4:5]
    kxy = smalls[:, 5:11].rearrange("p (c s) -> p c s", c=2)
    iota30 = smalls[:, 20:50]
    ones = smalls[:, 50:51]
    wvec_i = smalls[:, 51:54].bitcast(I32)
    koff_i = smalls[:, 54:57].bitcast(I32)

    stack = pool.tile([128, NPART], F32, tag="stack")
    nc.vector.memset(stack[:], 0.0)
    stv = stack[:].rearrange("p (s j) -> p s j", j=6)

    ce = nc.gpsimd if CHAIN_ON_GPSIMD else nc.vector

    # ---- box -> cell key per scale.  floor via round(x - 0.5) fused into
    # the i32-out cast; the reference's clamps are provably no-ops for
    # f32 coords in [0, 1): x*W never rounds up to W and round(x*W - 0.5)
    # stays within [0, W-1].  Key arithmetic in int32 (no float round-trip).
    gr = pool.tile([NP, 2, 3], F32, tag="gr")
    ce.tensor_tensor(
        out=gr[:], in0=btile[:, 0:2, None].to_broadcast([NP, 2, 3]), in1=kxy, op=OP.mult
    )
    gi = pool.tile([NP, 2, 3], I32, tag="gi")
    ce.tensor_scalar(out=gi[:], in0=gr[:], scalar1=-0.5, scalar2=None, op0=OP.add)
    keyi = pool.tile([NP, 3], I32, tag="keyi")
    ce.tensor_tensor(out=keyi[:], in0=gi[:, 1, :], in1=wvec_i, op=OP.mult)
    ce.tensor_add(keyi[:], keyi[:], gi[:, 0, :])
    ce.tensor_add(keyi[:], keyi[:], koff_i)
    keyf = pool.tile([NP, 3], F32, tag="keyf")
    ce.tensor_copy(out=keyf[:], in_=keyi[:])

    # ---- record gathers: 36-float row per (box, scale).  One gather per
    # scale: multi-offset-per-partition indirect DMAs generate garbled
    # addresses on hardware (verified empirically), so three ops it is.
    recg = pool.tile([NP, 3, RECW], F32, tag="recg")
    for s in range(3):
        nc.gpsimd.indirect_dma_start(
            out=recg[:, s, :],
            out_offset=None,
            in_=ins["rec"],
            in_offset=bass.IndirectOffsetOnAxis(ap=keyi[:, s : s + 1], axis=0),
        )

    if _DBG is not None:
        dbg, dbgk = _DBG
        nc.sync.dma_start(out=dbg, in_=recg[:].rearrange("p s r -> p (s r)"))
        nc.sync.dma_start(out=dbgk, in_=keyi[:])

    # ---- key/label row matrices: PE transpose of broadcast columns
    kl = kmps.tile([128, 512], F32, tag="kl")
    klv = kl[:].rearrange("p (s q) -> p s q", s=4)
    for s in range(3):
        nc.tensor.transpose(
            out=kl[:, 128 * s : 128 * (s + 1)],
            in_=keyf[:, s : s + 1].to_broadcast([128, 128]),
            identity=ident,
        )
    nc.tensor.transpose(out=kl[:, 384:512], in_=labB.to_broadcast([128, 128]), identity=ident)

    # ---- obj softplus over all cells: exp now, ln(1+x) with accum later
    obje = pool.tile([128, 132], F32, tag="obje")
    nc.scalar.activation(out=obje[:], in_=objd[:], func=AF.Exp)
    for s, (a, b) in enumerate([(0, 100), (100, 125), (125, 132)]):
        objl = pool.tile([128, b - a], F32, tag=f"objl{s}")
        nc.scalar.activation(
            out=objl[:], in_=obje[:, a:b], func=AF.Ln, bias=1.0,
            accum_out=stack[:, 6 * s + 4 : 6 * s + 5],
        )

    # ---- winners (last box per cell) + min same-cell label, all 3 scales.
    # win path first: the per-scale blocks need win3 for everything but the
    # cls column; minlab only gates the (late) selm select.
    eqm3 = pool.tile([128, 3, 128], F32, tag="eqm3")
    nc.vector.tensor_tensor(
        out=eqm3[:], in0=klv[:, 0:3, :], in1=keyf[:, :, None].to_broadcast([128, 3, 128]),
        op=OP.is_equal,
    )
    lose3 = pool.tile([128, 3, 128], F32, tag="lose3")
    nc.vector.tensor_tensor(
        out=lose3[:], in0=eqm3[:], in1=utri[:, None, :].to_broadcast([128, 3, 128]), op=OP.mult
    )
    losev = pool.tile([NP, 3], F32, tag="losev")
    nc.vector.tensor_reduce(out=losev[:], in_=lose3[:], axis=AX.X, op=OP.max)
    win3 = pool.tile([NP, 3], F32, tag="win3")
    nc.vector.tensor_scalar(
        out=win3[:], in0=losev[:], scalar1=-1.0, scalar2=1.0, op0=OP.mult, op1=OP.add
    )
    cnd3 = pool.tile([128, 3, 128], F32, tag="cnd3")
    nc.vector.scalar_tensor_tensor(
        out=cnd3[:], in0=eqm3[:], scalar=-BIGL,
        in1=kl[:, None, 384:512].to_broadcast([128, 3, 128]), op0=OP.mult, op1=OP.add,
    )
    minlab3 = pool.tile([NP, 3], F32, tag="minlab3")
    nc.vector.tensor_reduce(out=minlab3[:], in_=cnd3[:], axis=AX.X, op=OP.min)
    nc.vector.tensor_copy(out=stv[:, :, 5], in_=win3[:])  # npos columns

    # ---- per-scale post-processing, pipelined in the shadow of the next
    # scale's gather (each gather lands ~1.5us apart)
    expc = pool.tile([NP, 3, C], F32, tag="expc")
    sume = pool.tile([NP, 3], F32, tag="sume")
    lse3 = pool.tile([NP, 3], F32, tag="lse3")
    selm = pool.tile([NP, 3, C], F32, tag="selm")
    d12 = pool.tile([NP, 3, 4], F32, tag="d12")
    q12 = pool.tile([NP, 3, 4], F32, tag="q12")
    h12 = pool.tile([NP, 3, 4], F32, tag="h12")
    sl13 = pool.tile([NP, 3], F32, tag="sl13")
    clsv3 = pool.tile([NP, 3], F32, tag="clsv3")

    for s in range(3):
        rs = recg[:, s, :]
        wins = win3[:, s : s + 1]
        # ACT: logsumexp (exp with free-axis accum, then ln) and the
        # win-scaled stack columns that don't need DVE results (activation
        # with a per-partition scale AP does the masking multiply)
        nc.scalar.activation(
            out=expc[:, s, :], in_=rs[:, 5:35], func=AF.Exp,
            accum_out=sume[:, s : s + 1],
        )
        nc.scalar.activation(out=lse3[:, s : s + 1], in_=sume[:, s : s + 1], func=AF.Ln)
        # DVE: smooth-L1 (beta=1, coord mean, clamp 10)
        nc.vector.tensor_tensor(out=d12[:, s, :], in0=rs[:, 1:5], in1=btile, op=OP.subtract)
        nc.vector.scalar_tensor_tensor(
            out=d12[:, s, :], in0=d12[:, s, :], scalar=-1.0, in1=d12[:, s, :],
            op0=OP.mult, op1=OP.max,
        )
        nc.vector.tensor_scalar_min(q12[:, s, :], d12[:, s, :], 1.0)
        nc.vector.scalar_tensor_tensor(
            out=h12[:, s, :], in0=q12[:, s, :], scalar=-0.5, in1=d12[:, s, :],
            op0=OP.mult, op1=OP.add,
        )
        nc.vector.tensor_mul(h12[:, s, :], h12[:, s, :], q12[:, s, :])
        nc.vector.tensor_reduce(out=sl13[:, s : s + 1], in_=h12[:, s, :], axis=AX.X, op=OP.add)
        nc.vector.tensor_scalar(
            out=sl13[:, s : s + 1], in0=sl13[:, s : s + 1],
            scalar1=0.25, scalar2=10.0, op0=OP.mult, op1=OP.min,
        )
        # stack columns for this scale
        nc.vector.tensor_mul(stv[:, s, 0:1], lse3[:, s : s + 1], wins)
        nc.vector.tensor_mul(stv[:, s, 2:3], sl13[:, s : s + 1], wins)
        nc.vector.tensor_mul(stv[:, s, 3:4], rs[:, 0:1], wins)
        # cls logit at the min label: fused per-partition-scalar mask+mult
        nc.vector.scalar_tensor_tensor(
            out=selm[:, s, :], in0=iota30, scalar=minlab3[:, s : s + 1],
            in1=rs[:, 5:35], op0=OP.is_equal, op1=OP.mult,
        )
        nc.vector.tensor_reduce(out=clsv3[:, s : s + 1], in_=selm[:, s, :], axis=AX.X, op=OP.add)
        nc.vector.tensor_mul(stv[:, s, 1:2], clsv3[:, s : s + 1], wins)

    # ---- final: partition-reduce the stack on gpsimd (idle after gathers)
    # -> [1,18] -> single-descriptor DMA out
    if FIN_ON_GPSIMD:
        fin_sb = pool.tile([1, NPART], F32, tag="fin_sb")
        nc.gpsimd.tensor_reduce(out=fin_sb[:], in_=stack[:], axis=AX.C, op=OP.add)
    else:
        fin_ps = fips.tile([1, NPART], F32, tag="fin_ps")
        nc.tensor.matmul(out=fin_ps[:], lhsT=ones, rhs=stack[:], start=True, stop=True)
        fin_sb = pool.tile([1, NPART], F32, tag="fin_sb")
        nc.vector.tensor_copy(out=fin_sb[:], in_=fin_ps[:])
    nc.sync.dma_start(out=out_ap, in_=fin_sb[:])

    for p in reversed(pools):
        p.release()


def _patch_act_tables(nc):
    """Point every ACT table load at the combined exp+ln set and drop the
    redundant reloads (the greedy insertion pass ping-pongs between the
    exp-only and ln-only tables).  Loads are inserted after semaphore
    generation, so removal is safe."""
    tables = list(__import__("concourse.hw_specs", fromlist=["x"]).get_activation_tables(nc.m.arch).items())
    target = None
    for i, (name, funcs) in enumerate(tables):
        if AF.Exp in funcs and AF.Ln in funcs:
            target = i
            break
    if target is None:
        return
    first_seen = False
    for blk in nc.main_func.blocks:
        keep = []
        for inst in blk.instructions:
            if isinstance(inst, mybir.InstLoadActFuncSet):
                if not first_seen:
                    inst.act_func_set_id = target
                    first_seen = True
                    keep.append(inst)
                # drop later loads: one combined table serves every func
            else:
                keep.append(inst)
        blk.instructions[:] = keep


# ---------------------------------------------------------------------------
# host side
# ---------------------------------------------------------------------------

_CACHE = {}


def _build():
    if "nc" in _CACHE:
        return _CACHE["nc"]
    nc = bacc.Bacc(
        "TRN2",
        target_bir_lowering=False,
        debug=False,
        enable_asserts=False,
        num_devices=N_CORES,
    )
    bigt_h = nc.inline_tensor(_bigt_const(), name="cbig")
    ins = {
        "rec": nc.dram_tensor("rec", (NREC, RECW), F32, kind="ExternalInput").ap(),
        "objd": nc.dram_tensor("objd", (128, 132), F32, kind="ExternalInput").ap(),
        "smalls": nc.dram_tensor("smalls", (128, 64), F32, kind="ExternalInput").ap(),
        "bigt": bigt_h.ap(),
    }
    out = nc.dram_tensor("partials", (1, NPART), F32, kind="ExternalOutput").ap()

    with tile.TileContext(nc) as tc:
        emit(tc, out, ins)
    nc.compile()
    _patch_act_tables(nc)
    _CACHE["nc"] = nc
    return nc


def _prep_core(inputs, lo, hi):
    rec = np.zeros((NREC, RECW), np.float32)
    r0 = 0
    for s, (h, w) in enumerate(SCALES):
        hw = h * w
        n = B_SH * hw
        rec[r0 : r0 + n, 0] = np.asarray(inputs[f"obj_p{s}"][lo:hi]).reshape(n)
        rec[r0 : r0 + n, 1:5] = (
            np.asarray(inputs[f"reg_p{s}"][lo:hi]).reshape(B_SH, 4, hw).transpose(0, 2, 1).reshape(n, 4)
        )
        rec[r0 : r0 + n, 5:35] = (
            np.asarray(inputs[f"cls_p{s}"][lo:hi]).reshape(B_SH, C, hw).transpose(0, 2, 1).reshape(n, C)
        )
        r0 += n

    objd = np.empty((128, 132), np.float32)
    objd[:, 0:100] = np.asarray(inputs["obj_p0"][lo:hi]).reshape(128, 100)
    objd[:, 100:125] = np.asarray(inputs["obj_p1"][lo:hi]).reshape(128, 25)
    z = np.full(896, PADV, np.float32)
    z[:800] = np.asarray(inputs["obj_p2"][lo:hi]).reshape(800)
    objd[:, 125:132] = z.reshape(128, 7)

    smalls = np.zeros((128, 64), np.float32)
    smalls[:, 0:4] = np.asarray(inputs["boxes"][lo:hi]).reshape(128, 4)
    smalls[:, 4] = np.asarray(inputs["labels"][lo:hi]).reshape(128).astype(np.float32) + BIGL
    smalls[:, 5:57] = _SMALLS_KC
    return {"rec": rec, "objd": objd, "smalls": smalls}


def combine_partials(parts):
    """parts: [n_cores, 18] -> final [4] losses."""
    tot = np.asarray(parts, np.float64).sum(axis=0)
    cls_sum = reg_sum = obj_sum = 0.0
    for s, (h, w) in enumerate(SCALES):
        b = 6 * s
        lse, val, sl1, obj, sp, npos = tot[b : b + 6]
        npos = max(npos, 1.0)
        cls_sum += (lse - val) / npos * CLS_W
        reg_sum += sl1 / npos * REG_W
        obj_sum += (sp - obj) / (B_TOT * h * w) * OBJ_W
    cls_sum /= len(SCALES)
    reg_sum /= len(SCALES)
    obj_sum /= len(SCALES)
    total = cls_sum + reg_sum + obj_sum
    return np.array([total, cls_sum, reg_sum, obj_sum], np.float32)


TRACE = False
LAST_RESULT = None


def kernel(**inputs):
    global LAST_RESULT
    nc = _build()
    in_maps = [_prep_core(inputs, c * B_SH, (c + 1) * B_SH) for c in range(N_CORES)]
    res = run_bass_kernel_spmd(
        nc, in_maps, core_ids=list(range(N_CORES)), trace=TRACE
    )
    LAST_RESULT = res
    parts = np.stack([np.asarray(r["partials"]).reshape(NPART) for r in res.results])
    return combine_partials(parts)
